# Initial kernel scaffold
#
import sys
if '/opt/trn_rl_repo' not in sys.path:
    sys.path.insert(0, '/opt/trn_rl_repo')
import numpy as np

B, S, D, H, DH, F = 2, 2048, 1024, 16, 64, 4096
NQ = 512           # queries per core (tokens q with q % 4 == j, batch b = core//4)
NCORES = 8
P = 128
EPS = 1e-5
VW = 65            # per-head width in vv: 64 v cols + 1 ones col (denominator)
VT = 16 * VW       # cols per key-tile in vv = 1040
NT = 16            # key tiles of 128
NPAIR = 8          # key-tile pairs of 256
LOOKAHEAD = 2


def build_nc():
    import concourse.bass as bass
    import concourse.tile as tile
    from concourse import bacc, mybir

    f32 = mybir.dt.float32
    f32r = mybir.dt.float32r
    bf16 = mybir.dt.bfloat16
    fp8 = mybir.dt.float8e4
    AF = mybir.ActivationFunctionType
    OP = mybir.AluOpType
    DR = mybir.MatmulPerfMode.DoubleRow

    nc = bacc.Bacc("TRN2", target_bir_lowering=False, debug=False,
                   num_devices=NCORES)

    def din(name, shape, dt=f32):
        return nc.dram_tensor(name, shape, dt, kind="ExternalInput").ap()

    # host-pretransposed, per-partition-contiguous layouts
    x8_d = din("xb_pt", [P, 8 * S], bf16)       # [p, f, s] x^T bf16
    xq8_d = din("xqb_pt", [P, 8 * NQ], bf16)    # [p, f, q] striped query cols bf16
    xqT_d = din("xqT_pt", [P, 8 * NQ])          # [p, m, q] striped query cols f32
    wq8_d = din("wq_pt", [P, 8 * 8 * P], bf16)  # [p, m, f, c]
    wk8_d = din("wk_pt", [P, 8 * 8 * P], bf16)
    wv8_d = din("wv_pt", [P, 2 * 8 * NQ], bf16)  # [p, nh, f, c]
    wo_d = din("wo_pt", [P, 8 * 8 * P], bf16)   # [p, m, f, c]
    w1_d = din("w1_pt", [P, 32 * 8 * P], bf16)  # [p, mf, f, c]
    w2_d = din("w2_pt", [P, 8 * 32 * P], bf16)  # [p, m, kf, c]
    bq_d = din("bq_pt", [P, 8])
    bk_d = din("bk_pt", [P, 8])
    bv_d = din("bv_pt", [P, 8])
    bo_d = din("bo_pt", [P, 8])
    b1_d = din("b1_pt", [P, 32])
    b2_d = din("b2_pt", [P, 8])
    g1_d = din("g1_pt", [P, 8])
    be1_d = din("beta1_pt", [P, 8])
    g2_d = din("g2_pt", [P, 8])
    be2_d = din("beta2_pt", [P, 8])
    qm_d = din("qmask", [P, 2 * 64])            # additive causal boundary mask
    ones_d = din("onesmat", [P, P], f32r)
    outT_d = nc.dram_tensor("outT", [D, NQ], f32r, kind="ExternalOutput").ap()

    with tile.TileContext(nc) as tc:
        import contextlib
        with contextlib.ExitStack() as top:
            persist = top.enter_context(tc.tile_pool(name="persist", bufs=1))
            ones = persist.tile([P, P], f32r)
            nc.sync.dma_start(ones[:], ones_d[:])
            qmask = persist.tile([P, 2, 64], f32)
            nc.sync.dma_start(qmask[:], qm_d.rearrange("p (t c) -> p t c", t=2))
            biases = {}
            for nm, dr, w in [("bq", bq_d, 8), ("bk", bk_d, 8), ("bv", bv_d, 8),
                              ("bo", bo_d, 8), ("b1", b1_d, 32), ("b2", b2_d, 8),
                              ("g1", g1_d, 8), ("be1", be1_d, 8),
                              ("g2", g2_d, 8), ("be2", be2_d, 8)]:
                t = persist.tile([P, w], f32, name=f"bias_{nm}")
                nc.sync.dma_start(t[:], dr[:])
                biases[nm] = t

            # attention output (transposed [feat, query]) survives into phase 3
            outp = top.enter_context(tc.tile_pool(name="outp", bufs=1))
            outT = outp.tile([P, 8 * NQ], bf16)

            with contextlib.ExitStack() as ascope:
                st = ascope.enter_context(tc.tile_pool(name="attn_state", bufs=1))
                kT = st.tile([P, 8 * S], bf16)    # [feat-chunk, key], 8 chunks
                qT = st.tile([P, 8 * NQ], bf16)   # [feat-chunk, query]
                xr = ascope.enter_context(tc.tile_pool(name="xres_p", bufs=1))
                x8 = xr.tile([P, 8, S], bf16)     # x^T bf16, chunk-major
                xq8 = xr.tile([P, 8, NQ], bf16)

                # ---- phase 1a: kT + qT (bf16) ----
                with tc.tile_pool(name="w1a_p", bufs=4) as wp, \
                     tc.tile_pool(name="ps1a", bufs=4, space="PSUM") as pp:
                    wts = {}

                    def load_w(m):
                        wkt = wp.tile([P, 8, P], bf16, name="wkt", tag="w1a")
                        nc.sync.dma_start(
                            wkt[:], wk8_d[:, m * 8 * P:(m + 1) * 8 * P]
                            .rearrange("p (f c) -> p f c", f=8))
                        wqt = wp.tile([P, 8, P], bf16, name="wqt", tag="w1a")
                        nc.sync.dma_start(
                            wqt[:], wq8_d[:, m * 8 * P:(m + 1) * 8 * P]
                            .rearrange("p (f c) -> p f c", f=8))
                        wts[m] = (wkt, wqt)

                    # DMA order: m=0/1 weights + query cols first (small),
                    # then x in two key-halves so K can start on half 1.
                    load_w(0)
                    load_w(1)
                    nc.sync.dma_start(xq8[:],
                                      xq8_d.rearrange("p (f q) -> p f q", f=8))
                    xv = x8_d.rearrange("p (f s) -> p f s", f=8)
                    nc.sync.dma_start(x8[:, :, 0:S // 2], xv[:, :, 0:S // 2])
                    nc.sync.dma_start(x8[:, :, S // 2:S], xv[:, :, S // 2:S])
                    for m in range(8):
                        if m + 2 < 8:
                            load_w(m + 2)
                        wkt, wqt = wts.pop(m)
                        ps = pp.tile([P, NQ], f32, name="psq")
                        for fc in range(8):
                            nc.tensor.matmul(
                                ps[:], wqt[:, fc:fc + 1, :],
                                xq8[:, fc:fc + 1, :],
                                start=(fc == 0), stop=(fc == 7))
                        nc.vector.tensor_scalar(
                            qT[:, m * NQ:(m + 1) * NQ], ps[:],
                            biases["bq"][:, m:m + 1], None, OP.add)
                        for tq in range(4):
                            ps = pp.tile([P, NQ], f32, name="psk")
                            for fc in range(8):
                                nc.tensor.matmul(
                                    ps[:], wkt[:, fc:fc + 1, :],
                                    x8[:, fc:fc + 1,
                                       tq * NQ:(tq + 1) * NQ],
                                    start=(fc == 0), stop=(fc == 7))
                            nc.vector.tensor_scalar(
                                kT[:, m * S + tq * NQ: m * S + (tq + 1) * NQ],
                                ps[:], biases["bk"][:, m:m + 1], None, OP.add)

                # ---- phase 1b + scores, software-pipelined ----
                st2 = ascope.enter_context(tc.tile_pool(name="attn_state2", bufs=1))
                vv = st2.tile([P, NT * VT + VW], bf16)  # [key, head-major v|ones]
                apool = ascope.enter_context(
                    tc.tile_pool(name="attn_p", bufs=2 + (LOOKAHEAD + 1) * NPAIR))
                pps = ascope.enter_context(
                    tc.tile_pool(name="ps2s", bufs=2, space="PSUM"))

                attn_store = {h: {} for h in range(H)}

                def emit_pair(h, p):
                    # scores for key-tile pair p (keys 256p..256p+255), query
                    # cols 64p..511; boundary causal mask on first 32/64 cols.
                    pb = (h % 2) * 64
                    ht = h // 2
                    W = NQ - 64 * p
                    ps_s = pps.tile([P, 2, NQ], f32, name="ps_s")
                    for half in range(2):
                        kt = 2 * p + half
                        nc.tensor.matmul(
                            ps_s[:, half:half + 1, 0:W],
                            kT[pb:pb + 64, ht * S + kt * P: ht * S + (kt + 1) * P],
                            qT[pb:pb + 64, ht * NQ + 64 * p:(ht + 1) * NQ],
                            start=True, stop=True)
                    nc.vector.tensor_add(ps_s[:, :, 0:64], ps_s[:, :, 0:64],
                                         qmask[:])
                    attn = apool.tile([P, 2, NQ], bf16, name="attn")
                    nc.scalar.activation(attn[:, :, 0:W], ps_s[:, :, 0:W],
                                         AF.Exp, scale=0.125)
                    attn_store[h][p] = attn

                with tc.tile_pool(name="wv_p", bufs=2) as wvp, \
                     tc.tile_pool(name="ps1b", bufs=4, space="PSUM") as pp:
                    # ones columns (denominator accumulators)
                    for tt in range(NT):
                        oc = bass.AP(vv.tensor, vv.offset + tt * VT + 64,
                                     [list(vv.ap[0]), [VW, 16]])
                        nc.vector.memset(oc, 1.0)
                    i1b = 0
                    for nh in range(2):
                        wvh = wvp.tile([P, 8, NQ], bf16, name="wvh")
                        nc.sync.dma_start(
                            wvh[:], wv8_d[:, nh * 8 * NQ:(nh + 1) * 8 * NQ]
                            .rearrange("p (f c) -> p f c", f=8))
                        for tt in range(NT):
                            ps = pp.tile([P, NQ], f32, name="psv")
                            for fc in range(8):
                                nc.tensor.matmul(
                                    ps[:],
                                    x8[:, fc:fc + 1, tt * P:(tt + 1) * P],
                                    wvh[:, fc:fc + 1, :],
                                    start=(fc == 0), stop=(fc == 7))
                            # psum col c (v-feat nh*512+c) -> vv col 65*(c//64)+c%64
                            vap = bass.AP(vv.tensor,
                                          vv.offset + tt * VT + nh * 8 * VW,
                                          [list(vv.ap[0]), [VW, 8], [1, 64]])
                            nc.vector.tensor_copy(vap, ps[:])
                            if i1b < LOOKAHEAD * NPAIR:
                                emit_pair(i1b // NPAIR, i1b % NPAIR)
                            i1b += 1

                # ---- AV + normalize ----
                with tc.tile_pool(name="rec_p", bufs=3) as rpool, \
                     tc.tile_pool(name="ps2a", bufs=3, space="PSUM") as ppa, \
                     tc.tile_pool(name="ps2b", bufs=1, space="PSUM") as ppb:

                    def finish_head(h, ps_av):
                        pbp = (h % 2) * 64
                        htp = h // 2
                        lnd = rpool.tile([P, NQ], f32, name="lnd", tag="rb")
                        nc.scalar.activation(lnd[64:65, :], ps_av[64:65, :],
                                             AF.Ln)
                        recip = rpool.tile([P, NQ], f32r, name="recip", tag="rb")
                        nc.scalar.activation(recip[64:65, :], lnd[64:65, :],
                                             AF.Exp, scale=-1.0)
                        ps_b = ppb.tile([P, NQ], f32, name="ps_b")
                        nc.tensor.matmul(ps_b[0:64, :], ones[64:65, 0:64],
                                         recip[64:65, :], start=True, stop=True)
                        rb = rpool.tile([P, NQ], f32, name="rb", tag="rb")
                        nc.vector.tensor_copy(rb[0:64, :], ps_b[0:64, :])
                        oslice = outT[pbp:pbp + 64, htp * NQ:(htp + 1) * NQ]
                        nc.vector.tensor_mul(oslice, ps_av[0:64, :], rb[0:64, :])
                        nc.vector.tensor_scalar(
                            oslice, oslice,
                            biases["bv"][pbp:pbp + 64, htp:htp + 1],
                            None, OP.add)

                    for h in range(H):
                        ps_av = ppa.tile([P, NQ], f32, name="ps_av")
                        for p in range(NPAIR):
                            if h + LOOKAHEAD < H and p % 2 == 0:
                                emit_pair(h + LOOKAHEAD, p)
                                emit_pair(h + LOOKAHEAD, p + 1)
                            W = NQ - 64 * p
                            for half in range(2):
                                kt = 2 * p + half
                                lhsT = bass.AP(
                                    vv.tensor, vv.offset + kt * VT + h * VW,
                                    [list(vv.ap[0]), [1, VW]])
                                nc.tensor.matmul(
                                    ps_av[0:VW, 64 * p:NQ], lhsT,
                                    attn_store[h][p][:, half:half + 1, 0:W],
                                    start=(p == 0 and half == 0),
                                    stop=(p == NPAIR - 1 and half == 1),
                                    skip_group_check=True)
                        finish_head(h, ps_av)

            # ---- phase 3: O-proj + LN1 + FFN + LN2 ----
            with tc.tile_pool(name="f3", bufs=1) as f3, \
                 tc.tile_pool(name="w3_p", bufs=3) as wp3, \
                 tc.tile_pool(name="sq_p", bufs=2) as sqp, \
                 tc.tile_pool(name="stat_p", bufs=1) as stp, \
                 tc.tile_pool(name="psmm", bufs=3, space="PSUM") as pmm, \
                 tc.tile_pool(name="psst", bufs=1, space="PSUM") as pst, \
                 tc.tile_pool(name="psbc", bufs=2, space="PSUM") as pbc:
                xq = f3.tile([P, 8 * NQ], f32)
                nc.sync.dma_start(xq[:], xqT_d[:])
                x1 = f3.tile([P, 8 * NQ], f32r)
                x1b = f3.tile([P, 8 * NQ], bf16)
                hh = f3.tile([P, 32 * NQ], bf16)
                x2 = f3.tile([P, 8 * NQ], f32r)

                def ln_accum(xt, m, ps_sum, ps_sq):
                    # fold per-chunk LN stats into the producing loop
                    nc.tensor.matmul(ps_sum[:], ones[:, 0:1],
                                     xt[:, m * NQ:(m + 1) * NQ],
                                     start=(m == 0), stop=(m == 7),
                                     skip_group_check=True)
                    sq = sqp.tile([P, NQ], f32r, name="sq")
                    nc.scalar.activation(sq[:], xt[:, m * NQ:(m + 1) * NQ],
                                         AF.Square)
                    nc.tensor.matmul(ps_sq[:], ones[:, 0:1], sq[:],
                                     start=(m == 0), stop=(m == 7),
                                     skip_group_check=True)

                # O-projection + residual, LN1 stats interleaved
                ps_sum1 = pst.tile([1, NQ], f32, name="ps_sum1", tag="sum")
                ps_sq1 = pst.tile([1, NQ], f32, name="ps_sq1", tag="sumsq")
                for m in range(8):
                    wot = wp3.tile([P, 8, P], bf16, name="wot", tag="wsmall")
                    nc.sync.dma_start(
                        wot[:], wo_d[:, m * 8 * P:(m + 1) * 8 * P]
                        .rearrange("p (f c) -> p f c", f=8))
                    ps = pmm.tile([P, NQ], f32, name="psmm")
                    for f2 in range(8):
                        nc.tensor.matmul(
                            ps[:], wot[:, f2:f2 + 1, :],
                            outT[:, f2 * NQ:(f2 + 1) * NQ],
                            start=(f2 == 0), stop=(f2 == 7))
                    nc.vector.scalar_tensor_tensor(
                        x1[:, m * NQ:(m + 1) * NQ], ps[:],
                        biases["bo"][:, m:m + 1],
                        xq[:, m * NQ:(m + 1) * NQ], OP.add, OP.add)
                    ln_accum(x1, m, ps_sum1, ps_sq1)

                def layer_norm(xt, gname, bname, ps_sum, ps_sq):
                    mu = stp.tile([1, NQ], f32r, name="mu")
                    nc.scalar.activation(mu[:], ps_sum[:], AF.Copy,
                                         scale=1.0 / D)
                    ex2 = stp.tile([1, NQ], f32, name="ex2")
                    nc.scalar.activation(ex2[:], ps_sq[:], AF.Copy,
                                         scale=1.0 / D)
                    var = stp.tile([1, NQ], f32, name="var")
                    nc.vector.scalar_tensor_tensor(
                        var[:], mu[:], 0.0, mu[:], OP.add, OP.mult)
                    nc.vector.scalar_tensor_tensor(
                        var[:], ex2[:], EPS, var[:], OP.add, OP.subtract)
                    lnv = stp.tile([1, NQ], f32, name="lnv")
                    nc.scalar.activation(lnv[:], var[:], AF.Ln)
                    rstd = stp.tile([1, NQ], f32r, name="rstd")
                    nc.scalar.activation(rstd[:], lnv[:], AF.Exp, scale=-0.5)
                    ps_mu = pbc.tile([P, NQ], f32, name="ps_mu", tag="bc")
                    nc.tensor.matmul(ps_mu[:], ones[0:1, :], mu[:],
                                     start=True, stop=True)
                    ps_rs = pbc.tile([P, NQ], f32, name="ps_rs", tag="bc")
                    nc.tensor.matmul(ps_rs[:], ones[0:1, :], rstd[:],
                                     start=True, stop=True)
                    for m in range(8):
                        sl = xt[:, m * NQ:(m + 1) * NQ]
                        nc.vector.tensor_sub(sl, sl, ps_mu[:])
                        nc.vector.tensor_mul(sl, sl, ps_rs[:])
                        nc.scalar.activation(sl, sl, AF.Identity,
                                             bias=biases[bname][:, m:m + 1],
                                             scale=biases[gname][:, m:m + 1])

                layer_norm(x1, "g1", "be1", ps_sum1, ps_sq1)
                for m in range(8):
                    nc.scalar.copy(x1b[:, m * NQ:(m + 1) * NQ],
                                   x1[:, m * NQ:(m + 1) * NQ])

                # FFN1 (relu) -> hh
                for mf in range(32):
                    w1t = wp3.tile([P, 8, P], bf16, name="w1t", tag="wsmall")
                    nc.sync.dma_start(
                        w1t[:], w1_d[:, mf * 8 * P:(mf + 1) * 8 * P]
                        .rearrange("p (f c) -> p f c", f=8))
                    ps = pmm.tile([P, NQ], f32, name="psmm")
                    for f2 in range(8):
                        nc.tensor.matmul(
                            ps[:], w1t[:, f2:f2 + 1, :],
                            x1b[:, f2 * NQ:(f2 + 1) * NQ],
                            start=(f2 == 0), stop=(f2 == 7))
                    nc.scalar.activation(hh[:, mf * NQ:(mf + 1) * NQ], ps[:],
                                         AF.Relu, bias=biases["b1"][:, mf:mf + 1])

                # FFN2 + residual -> x2, LN2 stats interleaved
                ps_sum2 = pst.tile([1, NQ], f32, name="ps_sum2", tag="sum")
                ps_sq2 = pst.tile([1, NQ], f32, name="ps_sq2", tag="sumsq")
                for m in range(8):
                    w2t = wp3.tile([P, 32, P], bf16, name="w2t", tag="wbig",
                                   bufs=2)
                    nc.sync.dma_start(
                        w2t[:], w2_d[:, m * 32 * P:(m + 1) * 32 * P]
                        .rearrange("p (f c) -> p f c", f=32))
                    ps = pmm.tile([P, NQ], f32, name="psmm")
                    for kf in range(32):
                        nc.tensor.matmul(
                            ps[:], w2t[:, kf:kf + 1, :],
                            hh[:, kf * NQ:(kf + 1) * NQ],
                            start=(kf == 0), stop=(kf == 31))
                    nc.vector.scalar_tensor_tensor(
                        x2[:, m * NQ:(m + 1) * NQ], ps[:],
                        biases["b2"][:, m:m + 1],
                        x1[:, m * NQ:(m + 1) * NQ], OP.add, OP.add)
                    ln_accum(x2, m, ps_sum2, ps_sq2)

                layer_norm(x2, "g2", "be2", ps_sum2, ps_sq2)

                for m in range(8):
                    nc.sync.dma_start(outT_d[m * P:(m + 1) * P, :],
                                      x2[:, m * NQ:(m + 1) * NQ])

    nc.compile()
    return nc


_CACHE = {}


def make_in_maps(inputs):
    import ml_dtypes
    bf = ml_dtypes.bfloat16
    e4 = ml_dtypes.float8_e4m3
    x = np.asarray(inputs['x'], dtype=np.float32)

    shared = {}
    # weights, host-pretransposed chunk-major: dst[p, m, f, c] = w[f*128+p, m*128+c]
    for nm, dst in [("wq", "wq_pt"), ("wk", "wk_pt"), ("wo", "wo_pt")]:
        w = np.asarray(inputs[nm], np.float32)
        shared[dst] = np.ascontiguousarray(
            w.reshape(8, P, 8, P).transpose(1, 2, 0, 3).reshape(P, -1)
        ).astype(bf)
    # wv: [p, nh, f, c] with c spanning 512
    wv = np.asarray(inputs["wv"], np.float32)
    shared["wv_pt"] = np.ascontiguousarray(
        wv.reshape(8, P, 2, NQ).transpose(1, 2, 0, 3).reshape(P, -1)).astype(bf)
    w1 = np.asarray(inputs["w1"], np.float32)
    shared["w1_pt"] = np.ascontiguousarray(
        w1.reshape(8, P, 32, P).transpose(1, 2, 0, 3).reshape(P, -1)).astype(bf)
    w2 = np.asarray(inputs["w2"], np.float32)
    shared["w2_pt"] = np.ascontiguousarray(
        w2.reshape(32, P, 8, P).transpose(1, 2, 0, 3).reshape(P, -1)).astype(bf)

    for nm, w in [("bq", 8), ("bk", 8), ("bv", 8), ("bo", 8), ("b2", 8)]:
        shared[nm + "_pt"] = np.ascontiguousarray(
            np.asarray(inputs[nm], np.float32).reshape(w, P).T)
    shared["b1_pt"] = np.ascontiguousarray(
        np.asarray(inputs["b1"], np.float32).reshape(32, P).T)
    for src, dst in [("g1", "g1_pt"), ("beta1", "beta1_pt"),
                     ("g2", "g2_pt"), ("beta2", "beta2_pt")]:
        shared[dst] = np.ascontiguousarray(
            np.asarray(inputs[src], np.float32).reshape(8, P).T)
    shared["onesmat"] = np.ones((P, P), np.float32)

    in_maps = []
    for c in range(NCORES):
        b, j = c // 4, c % 4
        xb = x[b]                                # [S, D]
        qsel = np.arange(NQ) * 4 + j             # striped query tokens
        m = dict(shared)
        xT = xb.T                                # [D, S]
        m["xb_pt"] = np.ascontiguousarray(
            xT.reshape(8, P, S).transpose(1, 0, 2).reshape(P, -1)).astype(bf)
        m["xqb_pt"] = np.ascontiguousarray(
            xT[:, qsel].reshape(8, P, NQ).transpose(1, 0, 2).reshape(P, -1)
        ).astype(bf)
        m["xqT_pt"] = np.ascontiguousarray(
            xT[:, qsel].reshape(8, P, NQ).transpose(1, 0, 2).reshape(P, -1))
        # causal boundary mask [p, parity, 64]:
        #  even subtile (keys 256p+k): col ci<32 live iff k <= 4*ci+j
        #  odd  subtile (keys 256p+128+k): ci<32 dead; 32<=ci<64 live iff
        #    k <= 4*(ci-32)+j
        qm = np.zeros((P, 2, 64), np.float32)
        k = np.arange(P)[:, None]
        ci = np.arange(32)[None, :]
        qm[:, 0, 0:32] = np.where(k <= 4 * ci + j, 0.0, -1e4)
        qm[:, 1, 0:32] = -1e4
        qm[:, 1, 32:64] = np.where(k <= 4 * ci + j, 0.0, -1e4)
        m["qmask"] = np.ascontiguousarray(qm.reshape(P, -1))
        in_maps.append(m)
    return in_maps


def kernel(**inputs):
    from concourse.bass_utils import run_bass_kernel_spmd
    if "nc" not in _CACHE:
        _CACHE["nc"] = build_nc()
    nc = _CACHE["nc"]
    in_maps = make_in_maps(inputs)
    res = run_bass_kernel_spmd(nc, in_maps, core_ids=list(range(NCORES)))
    out = np.empty((B, S, D), np.float32)
    for c in range(NCORES):
        b, j = c // 4, c % 4
        qsel = np.arange(NQ) * 4 + j
        out[b, qsel, :] = res.results[c]["outT"].T
    return out



# revision 1
# speedup vs baseline: 1.0535x; 1.0535x over previous
import sys
if '/opt/trn_rl_repo' not in sys.path:
    sys.path.insert(0, '/opt/trn_rl_repo')
import numpy as np

B, S, D, H, DH, F = 2, 2048, 1024, 16, 64, 4096
NQ = 512           # queries per core (tokens q with q % 4 == j, batch b = core//4)
NCORES = 8
P = 128
EPS = 1e-5
VW = 65            # per-head width in vv: 64 v cols + 1 ones col (denominator)
VT = 16 * VW       # cols per key-tile in vv = 1040
NT = 16            # key tiles of 128
NPAIR = 8          # key-tile pairs of 256
LOOKAHEAD = 2


def build_nc():
    import concourse.bass as bass
    import concourse.tile as tile
    from concourse import bacc, mybir

    f32 = mybir.dt.float32
    f32r = mybir.dt.float32r
    bf16 = mybir.dt.bfloat16
    fp8 = mybir.dt.float8e4
    AF = mybir.ActivationFunctionType
    OP = mybir.AluOpType
    DR = mybir.MatmulPerfMode.DoubleRow

    nc = bacc.Bacc("TRN2", target_bir_lowering=False, debug=False,
                   num_devices=NCORES)

    def din(name, shape, dt=f32):
        return nc.dram_tensor(name, shape, dt, kind="ExternalInput").ap()

    # host-pretransposed, per-partition-contiguous layouts
    x8_d = din("xb_pt", [P, 8 * S], bf16)       # [p, f, s] x^T bf16
    xq8_d = din("xqb_pt", [P, 8 * NQ], bf16)    # [p, f, q] striped query cols bf16
    xqT_d = din("xqT_pt", [P, 8 * NQ])          # [p, m, q] striped query cols f32
    wq8_d = din("wq_pt", [P, 8 * 8 * P], bf16)  # [p, m, f, c]
    wk8_d = din("wk_pt", [P, 8 * 8 * P], bf16)
    wv8_d = din("wv_pt", [P, 2 * 8 * NQ], bf16)  # [p, nh, f, c]
    wo_d = din("wo_pt", [P, 8 * 8 * P], bf16)   # [p, m, f, c]
    w1_d = din("w1_pt", [P, 32 * 8 * P], bf16)  # [p, mf, f, c]
    w2_d = din("w2_pt", [P, 8 * 32 * P], bf16)  # [p, m, kf, c]
    bq_d = din("bq_pt", [P, 8])
    bk_d = din("bk_pt", [P, 8])
    bv_d = din("bv_pt", [P, 8])
    bo_d = din("bo_pt", [P, 8])
    b1_d = din("b1_pt", [P, 32])
    b2_d = din("b2_pt", [P, 8])
    g1_d = din("g1_pt", [P, 8])
    be1_d = din("beta1_pt", [P, 8])
    g2_d = din("g2_pt", [P, 8])
    be2_d = din("beta2_pt", [P, 8])
    qm_d = din("qmask", [P, 2 * 64])            # additive causal boundary mask
    ones_d = din("onesmat", [P, P], f32r)
    outT_d = nc.dram_tensor("outT", [D, NQ], f32r, kind="ExternalOutput").ap()

    with tile.TileContext(nc) as tc:
        import contextlib
        with contextlib.ExitStack() as top:
            persist = top.enter_context(tc.tile_pool(name="persist", bufs=1))
            ones = persist.tile([P, P], f32r)
            nc.sync.dma_start(ones[:], ones_d[:])
            qmask = persist.tile([P, 2, 64], f32)
            nc.sync.dma_start(qmask[:], qm_d.rearrange("p (t c) -> p t c", t=2))
            biases = {}
            for nm, dr, w in [("bq", bq_d, 8), ("bk", bk_d, 8), ("bv", bv_d, 8),
                              ("bo", bo_d, 8), ("b1", b1_d, 32), ("b2", b2_d, 8),
                              ("g1", g1_d, 8), ("be1", be1_d, 8),
                              ("g2", g2_d, 8), ("be2", be2_d, 8)]:
                t = persist.tile([P, w], f32, name=f"bias_{nm}")
                nc.sync.dma_start(t[:], dr[:])
                biases[nm] = t

            # attention output (transposed [feat, query]) survives into phase 3
            outp = top.enter_context(tc.tile_pool(name="outp", bufs=1))
            outT = outp.tile([P, 8 * NQ], bf16)

            with contextlib.ExitStack() as ascope:
                st = ascope.enter_context(tc.tile_pool(name="attn_state", bufs=1))
                kT = st.tile([P, 8 * S], bf16)    # [feat-chunk, key], 8 chunks
                qT = st.tile([P, 8 * NQ], bf16)   # [feat-chunk, query]
                xr = ascope.enter_context(tc.tile_pool(name="xres_p", bufs=1))
                x8 = xr.tile([P, 8, S], bf16)     # x^T bf16, chunk-major
                xq8 = xr.tile([P, 8, NQ], bf16)

                # ---- phase 1a: kT + qT (bf16) ----
                with tc.tile_pool(name="w1a_p", bufs=4) as wp, \
                     tc.tile_pool(name="ps1a", bufs=4, space="PSUM") as pp:
                    wts = {}

                    def load_w(m):
                        wkt = wp.tile([P, 8, P], bf16, name="wkt", tag="w1a")
                        nc.sync.dma_start(
                            wkt[:], wk8_d[:, m * 8 * P:(m + 1) * 8 * P]
                            .rearrange("p (f c) -> p f c", f=8))
                        wqt = wp.tile([P, 8, P], bf16, name="wqt", tag="w1a")
                        nc.sync.dma_start(
                            wqt[:], wq8_d[:, m * 8 * P:(m + 1) * 8 * P]
                            .rearrange("p (f c) -> p f c", f=8))
                        wts[m] = (wkt, wqt)

                    # DMA order: m=0/1 weights + query cols first (small),
                    # then x in two key-halves so K can start on half 1.
                    load_w(0)
                    load_w(1)
                    nc.sync.dma_start(xq8[:],
                                      xq8_d.rearrange("p (f q) -> p f q", f=8))
                    xv = x8_d.rearrange("p (f s) -> p f s", f=8)
                    nc.sync.dma_start(x8[:, :, 0:S // 2], xv[:, :, 0:S // 2])
                    nc.sync.dma_start(x8[:, :, S // 2:S], xv[:, :, S // 2:S])
                    for m in range(8):
                        if m + 2 < 8:
                            load_w(m + 2)
                        wkt, wqt = wts.pop(m)
                        ps = pp.tile([P, NQ], f32, name="psq")
                        for fc in range(8):
                            nc.tensor.matmul(
                                ps[:], wqt[:, fc:fc + 1, :],
                                xq8[:, fc:fc + 1, :],
                                start=(fc == 0), stop=(fc == 7))
                        nc.vector.tensor_scalar(
                            qT[:, m * NQ:(m + 1) * NQ], ps[:],
                            biases["bq"][:, m:m + 1], None, OP.add)
                        for tq in range(4):
                            ps = pp.tile([P, NQ], f32, name="psk")
                            for fc in range(8):
                                nc.tensor.matmul(
                                    ps[:], wkt[:, fc:fc + 1, :],
                                    x8[:, fc:fc + 1,
                                       tq * NQ:(tq + 1) * NQ],
                                    start=(fc == 0), stop=(fc == 7))
                            nc.vector.tensor_scalar(
                                kT[:, m * S + tq * NQ: m * S + (tq + 1) * NQ],
                                ps[:], biases["bk"][:, m:m + 1], None, OP.add)

                # ---- phase 1b + scores, software-pipelined ----
                st2 = ascope.enter_context(tc.tile_pool(name="attn_state2", bufs=1))
                vv = st2.tile([P, NT * VT + VW], bf16)  # [key, head-major v|ones]
                apool = ascope.enter_context(
                    tc.tile_pool(name="attn_p", bufs=2 + (LOOKAHEAD + 1) * NPAIR))
                pps = ascope.enter_context(
                    tc.tile_pool(name="ps2s", bufs=2, space="PSUM"))

                attn_store = {h: {} for h in range(H)}

                def emit_pair(h, p):
                    # scores for key-tile pair p (keys 256p..256p+255), query
                    # cols 64p..511; boundary causal mask on first 32/64 cols.
                    pb = (h % 2) * 64
                    ht = h // 2
                    W = NQ - 64 * p
                    ps_s = pps.tile([P, 2, NQ], f32, name="ps_s")
                    for half in range(2):
                        kt = 2 * p + half
                        nc.tensor.matmul(
                            ps_s[:, half:half + 1, 0:W],
                            kT[pb:pb + 64, ht * S + kt * P: ht * S + (kt + 1) * P],
                            qT[pb:pb + 64, ht * NQ + 64 * p:(ht + 1) * NQ],
                            start=True, stop=True)
                    nc.vector.tensor_add(ps_s[:, :, 0:64], ps_s[:, :, 0:64],
                                         qmask[:])
                    attn = apool.tile([P, 2, NQ], bf16, name="attn")
                    nc.scalar.activation(attn[:, :, 0:W], ps_s[:, :, 0:W],
                                         AF.Exp, scale=0.125)
                    attn_store[h][p] = attn

                with tc.tile_pool(name="wv_p", bufs=2) as wvp, \
                     tc.tile_pool(name="ps1b", bufs=4, space="PSUM") as pp:
                    # ones columns (denominator accumulators)
                    for tt in range(NT):
                        oc = bass.AP(vv.tensor, vv.offset + tt * VT + 64,
                                     [list(vv.ap[0]), [VW, 16]])
                        nc.vector.memset(oc, 1.0)
                    i1b = 0
                    for nh in range(2):
                        wvh = wvp.tile([P, 8, NQ], bf16, name="wvh")
                        nc.sync.dma_start(
                            wvh[:], wv8_d[:, nh * 8 * NQ:(nh + 1) * 8 * NQ]
                            .rearrange("p (f c) -> p f c", f=8))
                        for tt in range(NT):
                            ps = pp.tile([P, NQ], f32, name="psv")
                            for fc in range(8):
                                nc.tensor.matmul(
                                    ps[:],
                                    x8[:, fc:fc + 1, tt * P:(tt + 1) * P],
                                    wvh[:, fc:fc + 1, :],
                                    start=(fc == 0), stop=(fc == 7))
                            # psum col c (v-feat nh*512+c) -> vv col 65*(c//64)+c%64
                            vap = bass.AP(vv.tensor,
                                          vv.offset + tt * VT + nh * 8 * VW,
                                          [list(vv.ap[0]), [VW, 8], [1, 64]])
                            nc.vector.tensor_copy(vap, ps[:])
                            if i1b < LOOKAHEAD * NPAIR:
                                emit_pair(i1b // NPAIR, i1b % NPAIR)
                            i1b += 1

                # ---- AV + normalize ----
                with tc.tile_pool(name="rec_p", bufs=3) as rpool, \
                     tc.tile_pool(name="ps2a", bufs=3, space="PSUM") as ppa, \
                     tc.tile_pool(name="ps2b", bufs=1, space="PSUM") as ppb:

                    def finish_head(h, ps_av):
                        pbp = (h % 2) * 64
                        htp = h // 2
                        lnd = rpool.tile([P, NQ], f32, name="lnd", tag="rb")
                        nc.scalar.activation(lnd[64:65, :], ps_av[64:65, :],
                                             AF.Ln)
                        recip = rpool.tile([P, NQ], f32r, name="recip", tag="rb")
                        nc.scalar.activation(recip[64:65, :], lnd[64:65, :],
                                             AF.Exp, scale=-1.0)
                        ps_b = ppb.tile([P, NQ], f32, name="ps_b")
                        nc.tensor.matmul(ps_b[0:64, :], ones[64:65, 0:64],
                                         recip[64:65, :], start=True, stop=True)
                        rb = rpool.tile([P, NQ], f32, name="rb", tag="rb")
                        nc.vector.tensor_copy(rb[0:64, :], ps_b[0:64, :])
                        oslice = outT[pbp:pbp + 64, htp * NQ:(htp + 1) * NQ]
                        nc.vector.tensor_mul(oslice, ps_av[0:64, :], rb[0:64, :])
                        nc.vector.tensor_scalar(
                            oslice, oslice,
                            biases["bv"][pbp:pbp + 64, htp:htp + 1],
                            None, OP.add)

                    for h in range(H):
                        ps_av = ppa.tile([P, NQ], f32, name="ps_av")
                        for p in range(NPAIR):
                            if h + LOOKAHEAD < H and p % 2 == 0:
                                emit_pair(h + LOOKAHEAD, p)
                                emit_pair(h + LOOKAHEAD, p + 1)
                            W = NQ - 64 * p
                            for half in range(2):
                                kt = 2 * p + half
                                lhsT = bass.AP(
                                    vv.tensor, vv.offset + kt * VT + h * VW,
                                    [list(vv.ap[0]), [1, VW]])
                                nc.tensor.matmul(
                                    ps_av[0:VW, 64 * p:NQ], lhsT,
                                    attn_store[h][p][:, half:half + 1, 0:W],
                                    start=(p == 0 and half == 0),
                                    stop=(p == NPAIR - 1 and half == 1),
                                    skip_group_check=True)
                        finish_head(h, ps_av)

            # ---- phase 3: O-proj + LN1 + FFN + LN2 ----
            with tc.tile_pool(name="f3", bufs=1) as f3, \
                 tc.tile_pool(name="w3_p", bufs=3) as wp3, \
                 tc.tile_pool(name="sq_p", bufs=2) as sqp, \
                 tc.tile_pool(name="stat_p", bufs=1) as stp, \
                 tc.tile_pool(name="psmm", bufs=3, space="PSUM") as pmm, \
                 tc.tile_pool(name="psst", bufs=1, space="PSUM") as pst, \
                 tc.tile_pool(name="psbc", bufs=2, space="PSUM") as pbc:
                xq = f3.tile([P, 8 * NQ], f32)
                nc.sync.dma_start(xq[:], xqT_d[:])
                x1 = f3.tile([P, 8 * NQ], f32r)
                x1b = f3.tile([P, 8 * NQ], bf16)
                hh = f3.tile([P, 32 * NQ], bf16)
                x2 = f3.tile([P, 8 * NQ], f32r)

                def ln_accum(xt, m, ps_sum, ps_sq):
                    # fold per-chunk LN stats into the producing loop
                    nc.tensor.matmul(ps_sum[:], ones[:, 0:1],
                                     xt[:, m * NQ:(m + 1) * NQ],
                                     start=(m == 0), stop=(m == 7),
                                     skip_group_check=True)
                    sq = sqp.tile([P, NQ], f32r, name="sq")
                    nc.scalar.activation(sq[:], xt[:, m * NQ:(m + 1) * NQ],
                                         AF.Square)
                    nc.tensor.matmul(ps_sq[:], ones[:, 0:1], sq[:],
                                     start=(m == 0), stop=(m == 7),
                                     skip_group_check=True)

                # O-projection + residual, LN1 stats interleaved
                ps_sum1 = pst.tile([1, NQ], f32, name="ps_sum1", tag="sum")
                ps_sq1 = pst.tile([1, NQ], f32, name="ps_sq1", tag="sumsq")
                for m in range(8):
                    wot = wp3.tile([P, 8, P], bf16, name="wot", tag="wsmall")
                    nc.sync.dma_start(
                        wot[:], wo_d[:, m * 8 * P:(m + 1) * 8 * P]
                        .rearrange("p (f c) -> p f c", f=8))
                    ps = pmm.tile([P, NQ], f32, name="psmm")
                    for f2 in range(8):
                        nc.tensor.matmul(
                            ps[:], wot[:, f2:f2 + 1, :],
                            outT[:, f2 * NQ:(f2 + 1) * NQ],
                            start=(f2 == 0), stop=(f2 == 7))
                    nc.vector.scalar_tensor_tensor(
                        x1[:, m * NQ:(m + 1) * NQ], ps[:],
                        biases["bo"][:, m:m + 1],
                        xq[:, m * NQ:(m + 1) * NQ], OP.add, OP.add)
                    ln_accum(x1, m, ps_sum1, ps_sq1)

                def layer_norm(xt, gname, bname, ps_sum, ps_sq):
                    mu = stp.tile([1, NQ], f32r, name="mu")
                    nc.scalar.activation(mu[:], ps_sum[:], AF.Copy,
                                         scale=1.0 / D)
                    ex2 = stp.tile([1, NQ], f32, name="ex2")
                    nc.scalar.activation(ex2[:], ps_sq[:], AF.Copy,
                                         scale=1.0 / D)
                    var = stp.tile([1, NQ], f32, name="var")
                    nc.vector.scalar_tensor_tensor(
                        var[:], mu[:], 0.0, mu[:], OP.add, OP.mult)
                    nc.vector.scalar_tensor_tensor(
                        var[:], ex2[:], EPS, var[:], OP.add, OP.subtract)
                    lnv = stp.tile([1, NQ], f32, name="lnv")
                    nc.scalar.activation(lnv[:], var[:], AF.Ln)
                    rstd = stp.tile([1, NQ], f32r, name="rstd")
                    nc.scalar.activation(rstd[:], lnv[:], AF.Exp, scale=-0.5)
                    ps_mu = pbc.tile([P, NQ], f32, name="ps_mu", tag="bc")
                    nc.tensor.matmul(ps_mu[:], ones[0:1, :], mu[:],
                                     start=True, stop=True)
                    ps_rs = pbc.tile([P, NQ], f32, name="ps_rs", tag="bc")
                    nc.tensor.matmul(ps_rs[:], ones[0:1, :], rstd[:],
                                     start=True, stop=True)
                    for m in range(8):
                        sl = xt[:, m * NQ:(m + 1) * NQ]
                        nc.vector.tensor_sub(sl, sl, ps_mu[:])
                        nc.vector.tensor_mul(sl, sl, ps_rs[:])
                        nc.scalar.activation(sl, sl, AF.Identity,
                                             bias=biases[bname][:, m:m + 1],
                                             scale=biases[gname][:, m:m + 1])

                layer_norm(x1, "g1", "be1", ps_sum1, ps_sq1)
                for m in range(8):
                    nc.scalar.copy(x1b[:, m * NQ:(m + 1) * NQ],
                                   x1[:, m * NQ:(m + 1) * NQ])

                # FFN1 (relu) -> hh
                for mf in range(32):
                    w1t = wp3.tile([P, 8, P], bf16, name="w1t", tag="wsmall")
                    nc.sync.dma_start(
                        w1t[:], w1_d[:, mf * 8 * P:(mf + 1) * 8 * P]
                        .rearrange("p (f c) -> p f c", f=8))
                    ps = pmm.tile([P, NQ], f32, name="psmm")
                    for f2 in range(8):
                        nc.tensor.matmul(
                            ps[:], w1t[:, f2:f2 + 1, :],
                            x1b[:, f2 * NQ:(f2 + 1) * NQ],
                            start=(f2 == 0), stop=(f2 == 7))
                    nc.scalar.activation(hh[:, mf * NQ:(mf + 1) * NQ], ps[:],
                                         AF.Relu, bias=biases["b1"][:, mf:mf + 1])

                # FFN2 + residual -> x2, LN2 stats interleaved
                ps_sum2 = pst.tile([1, NQ], f32, name="ps_sum2", tag="sum")
                ps_sq2 = pst.tile([1, NQ], f32, name="ps_sq2", tag="sumsq")
                for m in range(8):
                    w2t = wp3.tile([P, 32, P], bf16, name="w2t", tag="wbig",
                                   bufs=2)
                    nc.sync.dma_start(
                        w2t[:], w2_d[:, m * 32 * P:(m + 1) * 32 * P]
                        .rearrange("p (f c) -> p f c", f=32))
                    ps = pmm.tile([P, NQ], f32, name="psmm")
                    for kf in range(32):
                        nc.tensor.matmul(
                            ps[:], w2t[:, kf:kf + 1, :],
                            hh[:, kf * NQ:(kf + 1) * NQ],
                            start=(kf == 0), stop=(kf == 31))
                    nc.vector.scalar_tensor_tensor(
                        x2[:, m * NQ:(m + 1) * NQ], ps[:],
                        biases["b2"][:, m:m + 1],
                        x1[:, m * NQ:(m + 1) * NQ], OP.add, OP.add)
                    ln_accum(x2, m, ps_sum2, ps_sq2)

                layer_norm(x2, "g2", "be2", ps_sum2, ps_sq2)

                for m in range(8):
                    nc.sync.dma_start(outT_d[m * P:(m + 1) * P, :],
                                      x2[:, m * NQ:(m + 1) * NQ])

    nc.compile()
    return nc


_CACHE = {}


def make_in_maps(inputs):
    import ml_dtypes
    bf = ml_dtypes.bfloat16
    e4 = ml_dtypes.float8_e4m3
    x = np.asarray(inputs['x'], dtype=np.float32)

    shared = {}
    # weights, host-pretransposed chunk-major: dst[p, m, f, c] = w[f*128+p, m*128+c]
    for nm, dst in [("wq", "wq_pt"), ("wk", "wk_pt"), ("wo", "wo_pt")]:
        w = np.asarray(inputs[nm], np.float32)
        shared[dst] = np.ascontiguousarray(
            w.reshape(8, P, 8, P).transpose(1, 2, 0, 3).reshape(P, -1)
        ).astype(bf)
    # wv: [p, nh, f, c] with c spanning 512
    wv = np.asarray(inputs["wv"], np.float32)
    shared["wv_pt"] = np.ascontiguousarray(
        wv.reshape(8, P, 2, NQ).transpose(1, 2, 0, 3).reshape(P, -1)).astype(bf)
    w1 = np.asarray(inputs["w1"], np.float32)
    shared["w1_pt"] = np.ascontiguousarray(
        w1.reshape(8, P, 32, P).transpose(1, 2, 0, 3).reshape(P, -1)).astype(bf)
    w2 = np.asarray(inputs["w2"], np.float32)
    shared["w2_pt"] = np.ascontiguousarray(
        w2.reshape(32, P, 8, P).transpose(1, 2, 0, 3).reshape(P, -1)).astype(bf)

    for nm, w in [("bq", 8), ("bk", 8), ("bv", 8), ("bo", 8), ("b2", 8)]:
        shared[nm + "_pt"] = np.ascontiguousarray(
            np.asarray(inputs[nm], np.float32).reshape(w, P).T)
    shared["b1_pt"] = np.ascontiguousarray(
        np.asarray(inputs["b1"], np.float32).reshape(32, P).T)
    for src, dst in [("g1", "g1_pt"), ("beta1", "beta1_pt"),
                     ("g2", "g2_pt"), ("beta2", "beta2_pt")]:
        shared[dst] = np.ascontiguousarray(
            np.asarray(inputs[src], np.float32).reshape(8, P).T)
    shared["onesmat"] = np.ones((P, P), np.float32)

    in_maps = []
    for c in range(NCORES):
        b, j = c // 4, c % 4
        xb = x[b]                                # [S, D]
        qsel = np.arange(NQ) * 4 + j             # striped query tokens
        m = dict(shared)
        xT = xb.T                                # [D, S]
        m["xb_pt"] = np.ascontiguousarray(
            xT.reshape(8, P, S).transpose(1, 0, 2).reshape(P, -1)).astype(bf)
        m["xqb_pt"] = np.ascontiguousarray(
            xT[:, qsel].reshape(8, P, NQ).transpose(1, 0, 2).reshape(P, -1)
        ).astype(bf)
        m["xqT_pt"] = np.ascontiguousarray(
            xT[:, qsel].reshape(8, P, NQ).transpose(1, 0, 2).reshape(P, -1))
        # causal boundary mask [p, parity, 64]:
        #  even subtile (keys 256p+k): col ci<32 live iff k <= 4*ci+j
        #  odd  subtile (keys 256p+128+k): ci<32 dead; 32<=ci<64 live iff
        #    k <= 4*(ci-32)+j
        qm = np.zeros((P, 2, 64), np.float32)
        k = np.arange(P)[:, None]
        ci = np.arange(32)[None, :]
        qm[:, 0, 0:32] = np.where(k <= 4 * ci + j, 0.0, -1e4)
        qm[:, 1, 0:32] = -1e4
        qm[:, 1, 32:64] = np.where(k <= 4 * ci + j, 0.0, -1e4)
        m["qmask"] = np.ascontiguousarray(qm.reshape(P, -1))
        in_maps.append(m)
    return in_maps


def kernel(**inputs):
    from concourse.bass_utils import run_bass_kernel_spmd
    if "nc" not in _CACHE:
        _CACHE["nc"] = build_nc()
    nc = _CACHE["nc"]
    in_maps = make_in_maps(inputs)
    res = run_bass_kernel_spmd(nc, in_maps, core_ids=list(range(NCORES)))
    out = np.empty((B, S, D), np.float32)
    for c in range(NCORES):
        b, j = c // 4, c % 4
        qsel = np.arange(NQ) * 4 + j
        out[b, qsel, :] = res.results[c]["outT"].T
    return out

